# revision 9
# baseline (speedup 1.0000x reference)
"""Trainium2 Bass kernel for nn_Attention (B=2, S=2048, D=2048, H=16, hd=128).

Sharding: 2-way batch DP x 4-way head TP over 8 cores.
Core c: batch b = c//4, head-group g = c%4 (heads 4g..4g+4).

Per-core pipeline (single SPMD program, per-core behavior via input data only):
  Per s-quarter q (512 rows):
    Phase 1: QKV projections from pre-transposed x (x^T in HBM), RoPE applied
             to Q^T/K^T in [hd, S] layout. The hd axis of q/k weights is
             host-permuted (even indices first) so RoPE pairs become partition
             halves (i, 64+i); scores are invariant to a shared q/k hd-perm.
             All of wq/wk/wv stay SBUF-resident (loaded once per run).
    Phase 2: causal attention for q-tile q, all heads: scores computed
             TRANSPOSED (sT[k, q-tile] = K^T.T @ Q^T), mask on diag blocks,
             exp on ACT, row-sums via ones-matmul, PV matmul -> attn^T[hd, q],
             normalized by 1/l broadcast (K=1 ones matmul).
  AllGather attn^T shards within each batch group of 4 cores.
  Phase 3: out-proj slice: out[:, 512 cols of this group] from full attn^T.

All matmuls use f16 operands (full-rate path on the PE).

build(reps=R>1) emits the complete per-run body R times in straight-line
code (collectives must stay straight-line for NRT, so no hardware loop).
Every DMA load, matmul, and AllGather re-executes per rep; test.py divides
steady-state per-dispatch time by R to amortize axon dispatch overhead.
"""

import math
import sys

import numpy as np

for _p in ("/opt/trn_rl_repo",):
    if _p not in sys.path:
        sys.path.insert(0, _p)

import concourse.bass as bass
import concourse.mybir as mybir
from concourse import bacc
from concourse.tile import TileContext

B, S, D, H, HD = 2, 2048, 2048, 16, 128
NC_TOTAL = 8
TPG = 4                 # head-TP group size
HPC = H // TPG          # heads per core = 4
P = 128
NDC = D // P            # 16 contraction chunks
ST = 512                # s/q tile width
NST = S // ST           # 4

f32 = mybir.dt.float32
f32r = mybir.dt.float32r
f16 = mybir.dt.float16
AF = mybir.ActivationFunctionType
ALU = mybir.AluOpType

_NC_CACHE = {}


def build(sim_single_core: bool = False, null_kernel: bool = False,
          reps: int = 1) -> bass.Bass:
    """null_kernel=True: same I/O signature + collective, ~zero compute.
    Used to measure the axon dispatch floor for timing by difference.

    reps>1: emit the whole per-run body `reps` times back-to-back
    (straight-line; collectives stay in NRT-required launch order). Each
    rep redoes every DMA load, matmul, and collective, so per-rep time is
    the true steady-state kernel time; only host/axon dispatch overhead is
    amortized. Used by test.py for timing."""
    nc = bacc.Bacc("TRN2", target_bir_lowering=False, debug=False,
                   num_devices=NC_TOTAL)

    xt = nc.declare_dram_parameter("xt", [D, S], f16, isOutput=False)
    wq_t = nc.declare_dram_parameter("wq_t", [D, HPC * HD], f16, isOutput=False)
    wk_t = nc.declare_dram_parameter("wk_t", [D, HPC * HD], f16, isOutput=False)
    wv_t = nc.declare_dram_parameter("wv_t", [D, HPC * HD], f16, isOutput=False)
    wo_t = nc.declare_dram_parameter("wo_t", [D, ST], f16, isOutput=False)
    cs_lo = nc.declare_dram_parameter("cs_lo", [64, S], f32, isOutput=False)
    sn_ng = nc.declare_dram_parameter("sn_ng", [64, S], f32, isOutput=False)
    mb = nc.declare_dram_parameter("mb", [P, 4, ST], f32, isOutput=False)
    out = nc.declare_dram_parameter("out", [S, ST], f32, isOutput=True)

    if null_kernel:
        with TileContext(nc) as tc:
            with (
                tc.tile_pool(name="sb", bufs=1) as sb,
                tc.tile_pool(name="dram", bufs=1, space="DRAM") as dpool,
            ):
                cc_in = dpool.tile([HPC * HD, ST], f16)
                cc_out = dpool.tile([D, ST], f16)
                t = sb.tile([P, ST], f16)
                nc.sync.dma_start(t[:], xt[0:P, 0:ST])
                nc.sync.dma_start(cc_in[0:P, :], t[:])
                nc.gpsimd.collective_compute(
                    "AllGather", ALU.bypass,
                    replica_groups=[[0, 1, 2, 3], [4, 5, 6, 7]],
                    ins=[cc_in[:]], outs=[cc_out[:]])
                t2 = sb.tile([P, ST], f32)
                nc.vector.tensor_copy(t2[:], t[:])
                nc.sync.dma_start(out[0:P, :], t2[:])
        nc.compile()
        return nc

    with TileContext(nc) as tc:
        with (
            tc.tile_pool(name="const", bufs=1) as cpool,
            tc.tile_pool(name="big", bufs=1) as big,
            tc.tile_pool(name="ps", bufs=1, space="PSUM") as ps,
            tc.tile_pool(name="dram", bufs=1, space="DRAM") as dpool,
        ):
            # ---- constants / persistent ----
            cs_sb = cpool.tile([64, S], f32)
            sn_sb = cpool.tile([64, S], f32)
            mb_sb = cpool.tile([P, 4, ST], f32)
            ones_col = cpool.tile([P, 1], f16)
            ones_row = cpool.tile([1, P], f32r)
            ones_f = cpool.tile([P, 1], f32)
            onesr_f = cpool.tile([1, P], f32)
            wo_sb = cpool.tile([P, NDC, ST], f16)
            # qkv weights SBUF-resident: loaded once per run (16KB/part each)
            # instead of re-streaming 32MB/run from HBM.
            wq_sb = cpool.tile([P, NDC, HPC * HD], f16)
            wk_sb = cpool.tile([P, NDC, HPC * HD], f16)
            wv_sb = cpool.tile([P, NDC, HPC * HD], f16)

            kt_all = big.tile([P, HPC, S], f16)           # K^T (rope'd, perm)
            qt_all = big.tile([P, HPC, S], f16)           # Q^T (rope'd, perm)
            v_all = big.tile([P, S // P, HPC * HD], f16)  # [s%128, s//128, h*hd]

            # Per-quarter collective buffers: AllGather_q launches right after
            # quarter q's attention; phase 3 for its s-tiles follows, all
            # overlapped with later quarters' compute.
            cc_in_q = [[dpool.tile([2 * HD, ST], f16, name=f"cc_in{j}_{p}")
                        for p in range(2)] for j in range(NST)]
            cc_out_q = [[dpool.tile([D // 2, ST], f16, name=f"cc_out{j}_{p}")
                         for p in range(2)] for j in range(NST)]

            def one_run(p12):
                nc.vector.memset(ones_f[:], 1.0)
                nc.vector.memset(onesr_f[:], 1.0)
                nc.vector.tensor_copy(ones_col[:], ones_f[:])
                nc.vector.tensor_copy(ones_row[:], onesr_f[:])

                if True:

                    def rope_from_psum(dst, qk_ps, s0):
                        """RoPE in [hd, ST] layout; pairs are partitions
                        (i, 64+i). dst/qk_ps = [128, ST]; s0 = global s."""
                        ssl = slice(s0, s0 + ST)
                        a_t = p12.tile([64, ST], f32, tag="rt", bufs=2,
                                       name="rt_a")
                        u_t = p12.tile([64, ST], f32, tag="rt", bufs=2,
                                       name="rt_u")
                        nc.vector.tensor_tensor(
                            a_t[:], qk_ps[0:64, :], cs_sb[:, ssl], ALU.mult)
                        nc.vector.tensor_tensor(
                            u_t[:], qk_ps[64:128, :], sn_sb[:, ssl], ALU.mult)
                        nc.vector.tensor_tensor(
                            dst[0:64, :], a_t[:], u_t[:], ALU.add)
                        a_b = p12.tile([64, ST], f32, tag="rt", bufs=2,
                                       name="rt_ab")
                        u_b = p12.tile([64, ST], f32, tag="rt", bufs=2,
                                       name="rt_ub")
                        nc.vector.tensor_tensor(
                            a_b[:], qk_ps[64:128, :], cs_sb[:, ssl], ALU.mult)
                        nc.vector.tensor_tensor(
                            u_b[:], qk_ps[0:64, :], sn_sb[:, ssl], ALU.mult)
                        nc.vector.tensor_tensor(
                            dst[64:128, :], a_b[:], u_b[:], ALU.subtract)

                    for q in range(NST):
                        s0 = q * ST
                        # ---------- phase 1 (s-quarter q) ----------
                        xt_q = p12.tile([P, NDC, ST], f16, tag="xtq", bufs=2,
                                        name="xt_q")
                        for dg in range(8):
                            nc.sync.dma_start(
                                xt_q[:, dg * 2:(dg + 1) * 2, :],
                                xt[dg * 2 * P:(dg + 1) * 2 * P,
                                   s0:s0 + ST].rearrange("(o p) s -> p o s",
                                                         p=P))
                        if q == 0:
                            # qkv weights: needed by this quarter's matmuls
                            for dg in range(4):
                                sl = slice(dg * 4 * P, (dg + 1) * 4 * P)
                                nc.sync.dma_start(
                                    wq_sb[:, dg * 4:(dg + 1) * 4, :],
                                    wq_t[sl, :].rearrange("(o p) f -> p o f",
                                                          p=P))
                                nc.sync.dma_start(
                                    wk_sb[:, dg * 4:(dg + 1) * 4, :],
                                    wk_t[sl, :].rearrange("(o p) f -> p o f",
                                                          p=P))
                                nc.sync.dma_start(
                                    wv_sb[:, dg * 4:(dg + 1) * 4, :],
                                    wv_t[sl, :].rearrange("(o p) f -> p o f",
                                                          p=P))
                            # consts are needed later than x/wv; load them
                            # after the critical-path streams.
                            nc.sync.dma_start(cs_sb[:], cs_lo[:])
                            nc.sync.dma_start(sn_sb[:], sn_ng[:])
                            nc.sync.dma_start(mb_sb[:], mb[:])
                            for dg in range(4):
                                nc.sync.dma_start(
                                    wo_sb[:, dg * 4:(dg + 1) * 4, :],
                                    wo_t[dg * 4 * P:(dg + 1) * 4 * P,
                                         :].rearrange("(o p) f -> p o f", p=P))

                        # V for the 4 s-chunks of this quarter
                        for vs in range(2):
                            v_ps = [
                                ps.tile([P, HPC * HD], f32, tag="vps", bufs=2,
                                        name=f"vps_{vs}_{i}")
                                for i in range(2)
                            ]
                            for dc in range(NDC):
                                for i in range(2):
                                    sc = vs * 2 + i
                                    nc.tensor.matmul(
                                        v_ps[i][:],
                                        xt_q[:, dc, sc * P:(sc + 1) * P],
                                        wv_sb[:, dc, :],
                                        start=(dc == 0), stop=(dc == NDC - 1),
                                        skip_group_check=True,
                                    )
                            for i in range(2):
                                nc.vector.tensor_copy(
                                    v_all[:, q * 4 + vs * 2 + i, :],
                                    v_ps[i][:])

                        # Q^T / K^T for this quarter with RoPE
                        for h in range(HPC):
                            hsl = slice(h * HD, (h + 1) * HD)
                            qt_ps = ps.tile([P, ST], f32, tag="qk", bufs=3,
                                            name="qt_ps")
                            for dc in range(NDC):
                                nc.tensor.matmul(
                                    qt_ps[:], wq_sb[:, dc, hsl],
                                    xt_q[:, dc, :],
                                    start=(dc == 0), stop=(dc == NDC - 1),
                                    skip_group_check=True,
                                )
                            rope_from_psum(qt_all[:, h, s0:s0 + ST], qt_ps, s0)
                            kt_ps = ps.tile([P, ST], f32, tag="qk", bufs=3,
                                            name="kt_ps")
                            for dc in range(NDC):
                                nc.tensor.matmul(
                                    kt_ps[:], wk_sb[:, dc, hsl],
                                    xt_q[:, dc, :],
                                    start=(dc == 0), stop=(dc == NDC - 1),
                                    skip_group_check=True,
                                )
                            rope_from_psum(kt_all[:, h, s0:s0 + ST], kt_ps, s0)

                        # ---------- phase 2 (q-tile q, all heads) ----------
                        kcs = 4 * q + 4      # causal: key chunks 0..kcs-1
                        for h in range(HPC):
                            l_ps = ps.tile([1, ST], f32, tag="lob", bufs=2,
                                           name="l_ps")
                            o_ps = ps.tile([P, ST], f32, tag="lob", bufs=2,
                                           name="o_ps")
                            prev_pt = None
                            for kc in range(kcs):
                                st_ps = ps.tile([P, ST], f32, tag="qk", bufs=3,
                                                name="st_ps")
                                nc.tensor.matmul(
                                    st_ps[:],
                                    kt_all[:, h, kc * P:(kc + 1) * P],
                                    qt_all[:, h, s0:s0 + ST],
                                    start=True, stop=True,
                                    skip_group_check=True,
                                )
                                if kc >= 4 * q:   # diagonal block: mask
                                    nc.vector.tensor_tensor(
                                        st_ps[:], st_ps[:],
                                        mb_sb[:, kc - 4 * q, :], ALU.add)
                                pt_sb = p12.tile([P, ST], f16, tag="pt",
                                                 bufs=5, name="pt_sb")
                                nc.scalar.activation(pt_sb[:], st_ps[:],
                                                     AF.Exp)
                                # 4-way tree PT reduction: DVE sums groups of
                                # 4 chunks; PE ones-matmul on group sums only
                                if kc % 2 == 0:
                                    prev_pt = pt_sb
                                else:
                                    pair = p12.tile([P, ST], f16, tag="pr",
                                                    bufs=5, name="pair")
                                    nc.vector.tensor_tensor(
                                        pair[:], prev_pt[:], pt_sb[:],
                                        ALU.add)
                                    if kc % 4 == 1:
                                        prev_pair = pair
                                    else:
                                        quad = p12.tile([P, ST], f16,
                                                        tag="pr", bufs=5,
                                                        name="quad")
                                        nc.vector.tensor_tensor(
                                            quad[:], prev_pair[:], pair[:],
                                            ALU.add)
                                        nc.tensor.matmul(
                                            l_ps[:], ones_col[:], quad[:],
                                            start=(kc == 3),
                                            stop=(kc == kcs - 1),
                                            skip_group_check=True,
                                        )
                                nc.tensor.matmul(
                                    o_ps[:],
                                    v_all[:, kc, h * HD:(h + 1) * HD],
                                    pt_sb[:],
                                    start=(kc == 0), stop=(kc == kcs - 1),
                                    skip_group_check=True,
                                )
                            recip = p12.tile([1, ST], f32r, tag="rcp", bufs=2,
                                             name="recip")
                            with nc.allow_low_precision(
                                    reason="1/l rounded to f32r for bcast"):
                                nc.vector.reciprocal(recip[:], l_ps[:])
                            bc_ps = ps.tile([P, ST], f32, tag="lob", bufs=2,
                                            name="bc_ps")
                            nc.tensor.matmul(
                                bc_ps[:], ones_row[:], recip[:],
                                start=True, stop=True, skip_group_check=True,
                            )
                            bc_sb = p12.tile([P, ST], f32, tag="bcs", bufs=2,
                                             name="bc_sb")
                            nc.vector.tensor_copy(bc_sb[:], bc_ps[:])
                            at_sb = p12.tile([P, ST], f16, tag="at", bufs=2,
                                             name="at_sb")
                            nc.vector.tensor_tensor(
                                at_sb[:], o_ps[:], bc_sb[:], ALU.mult)
                            nc.sync.dma_start(
                                cc_in_q[q][h // 2][(h % 2) * P:
                                                   (h % 2 + 1) * P, :],
                                at_sb[:])

                            if not sim_single_core and h % 2 == 1:
                                nc.gpsimd.collective_compute(
                                    "AllGather", ALU.bypass,
                                    replica_groups=[[0, 1, 2, 3],
                                                    [4, 5, 6, 7]],
                                    ins=[cc_in_q[q][h // 2][:]],
                                    outs=[cc_out_q[q][h // 2][:]])

                        # ---------- sim-mode collective stand-in ----------
                        if sim_single_core:
                            for pc in range(2):
                                for hh in range(2):
                                    tmp = p12.tile([P, ST], f16, tag="cc",
                                                   bufs=2, name="cc_tmp")
                                    nc.sync.dma_start(
                                        tmp[:],
                                        cc_in_q[q][pc][hh * P:(hh + 1) * P,
                                                       :])
                                    nc.sync.dma_start(
                                        cc_out_q[q][pc][hh * P:(hh + 1) * P,
                                                        :],
                                        tmp[:])
                                zz = p12.tile([P, ST], f16, tag="cc", bufs=2,
                                              name="zz")
                                nc.vector.memset(zz[:], 0.0)
                                for r in range(2 * HD, D // 2, P):
                                    nc.sync.dma_start(
                                        cc_out_q[q][pc][r:r + P, :], zz[:])

                        for st in range(4 * q, 4 * q + 4):
                            c0 = (st % 4) * P
                            a_sb = p12.tile([P, NDC, P], f16, tag="acc",
                                            bufs=2, name="a_sb")
                            for pc in range(2):
                                for r in range(4):
                                    nc.sync.dma_start(
                                        a_sb[:, 4 * r + 2 * pc:
                                             4 * r + 2 * pc + 2, :],
                                        cc_out_q[q][pc][
                                            r * 2 * P:(r + 1) * 2 * P,
                                            c0:c0 + P].rearrange(
                                            "(o p) f -> p o f", p=P),
                                    )
                            o3_ps = ps.tile([P, ST], f32, tag="o3", bufs=1,
                                            name="o3_ps")
                            # piece-0 chunks first: they only depend on the
                            # first AllGather of this quarter, so they can
                            # run while the second is still in flight.
                            dcs = [4 * r + 2 * pc + i
                                   for pc in range(2) for r in range(4)
                                   for i in range(2)]
                            for n_i, dc in enumerate(dcs):
                                nc.tensor.matmul(
                                    o3_ps[:], a_sb[:, dc, :], wo_sb[:, dc, :],
                                    start=(n_i == 0), stop=(n_i == NDC - 1),
                                    skip_group_check=True,
                                )
                            o3_sb = p12.tile([P, ST], f32, tag="o3s", bufs=2,
                                             name="o3_sb")
                            nc.vector.tensor_copy(o3_sb[:], o3_ps[:])
                            nc.sync.dma_start(out[st * P:(st + 1) * P, :],
                                              o3_sb[:])

            with tc.tile_pool(name="p12", bufs=1) as p12:
                for _ in range(reps):
                    one_run(p12)

    nc.compile()
    return nc


def _get_nc(sim_single_core: bool = False) -> bass.Bass:
    key = bool(sim_single_core)
    if key not in _NC_CACHE:
        _NC_CACHE[key] = build(sim_single_core)
    return _NC_CACHE[key]


def make_core_inputs(x, freqs_cos, freqs_sin, mask, w_in, w_out):
    """Host-side sharding/layout prep. Returns list of 8 per-core input dicts."""
    x = np.asarray(x, np.float32)
    freqs_cos = np.asarray(freqs_cos, np.float32)
    freqs_sin = np.asarray(freqs_sin, np.float32)
    mask = np.asarray(mask, np.float32)
    w_in = np.asarray(w_in, np.float32)
    w_out = np.asarray(w_out, np.float32)

    perm = np.concatenate([np.arange(0, HD, 2), np.arange(1, HD, 2)])
    cs_lo = np.ascontiguousarray(freqs_cos.T)               # [64, S]
    sn_ng = np.ascontiguousarray(-freqs_sin.T)              # [64, S]
    mb = np.ascontiguousarray(
        mask[:ST, :ST].T.reshape(4, P, ST).transpose(1, 0, 2))   # [128, 4, 512]
    xt_b = [np.ascontiguousarray(x[b].T).astype(np.float16) for b in range(B)]
    wo_T = np.ascontiguousarray(w_out.T)                     # [D, D]

    scale = 1.0 / math.sqrt(HD)
    in_maps = []
    for c in range(NC_TOTAL):
        b, g = c // TPG, c % TPG
        heads = range(g * HPC, (g + 1) * HPC)
        wq = np.vstack([w_in[h * HD:(h + 1) * HD][perm] for h in heads]) * scale
        wk = np.vstack([w_in[D + h * HD:D + (h + 1) * HD][perm] for h in heads])
        wv = np.vstack([w_in[2 * D + h * HD:2 * D + (h + 1) * HD] for h in heads])
        in_maps.append({
            "xt": xt_b[b],
            "wq_t": np.ascontiguousarray(wq.T).astype(np.float16),
            "wk_t": np.ascontiguousarray(wk.T).astype(np.float16),
            "wv_t": np.ascontiguousarray(wv.T).astype(np.float16),
            "wo_t": np.ascontiguousarray(wo_T[:, g * ST:(g + 1) * ST]).astype(np.float16),
            "cs_lo": cs_lo,
            "sn_ng": sn_ng,
            "mb": mb,
        })
    return in_maps


def run_spmd(inputs: dict, trace: bool = False):
    """Compile+run on cores 0-7. Returns (full_output, BassKernelResults)."""
    from concourse.bass_utils import run_bass_kernel_spmd

    in_maps = make_core_inputs(**inputs)
    nc = _get_nc(False)
    res = run_bass_kernel_spmd(nc, in_maps, list(range(NC_TOTAL)), trace=trace)
    out_full = np.empty((B, S, D), np.float32)
    for c in range(NC_TOTAL):
        b, g = c // TPG, c % TPG
        out_full[b, :, g * ST:(g + 1) * ST] = res.results[c]["out"]
    return out_full, res


def kernel(x, freqs_cos, freqs_sin, mask, w_in, w_out):
    out, _ = run_spmd(
        dict(x=x, freqs_cos=freqs_cos, freqs_sin=freqs_sin, mask=mask,
             w_in=w_in, w_out=w_out))
    return out
